# revision 33
# baseline (speedup 1.0000x reference)
"""Causal single-head attention on 8 Trainium2 NeuronCores.

Shapes (hardcoded per problem spec):
  input_tensor [512, 256, 384] f32, Wq/Wk/Wv [384, 64] f32 -> out [512, 256, 64] f32

Sharding: data-parallel on the batch dim, 64 batches per core, weights
replicated.

Per-batch-pair pipeline on each core (S=256 split into two 128-row blocks,
E=384 split into three 128-row chunks, GB=2 batches per group):
  1. DMA x pair [2,256,384] into SBUF with an f32->f16 cast (SWDGE).
  2. PE-transpose the twelve 128x128 blocks -> xT (f16 PSUM), DVE copies to
     SBUF in two pieces (chunks 0-1, then chunk 2) so the projections can
     start after the first piece.
  3. One PSUM tile [128,1024] holds both projections: [Wk|Wv] -> cols
     0:512 (kT at partitions 0:64, vT at 64:128), [Wq|0pad] -> cols
     512:1024 (the zero-pad makes the weights 128 wide so fast-weight-load
     engages).  DVE casts kv then q to f16 SBUF (split so the v-transposes
     unblock after the first).
  4. PE-transpose vT back to natural v [256,64]; ACT copies it beside a
     ones column (col 64 -> softmax denominator inside the AV matmul).
  5. Both batches' scores (f16 operands) in one PSUM tile [128,768], layout
     [b0k0(0:256) | b0k1(256:384) | b1k1(384:512) | b1k0(512:768)] so every
     matmul stays inside a 2KB PSUM bank.  start=True clears has_written
     for the WHOLE bank, so only the first matmul per bank opens; the rest
     accumulate/overwrite per element with start=False.
  6. Causal mask as PE accumulate: st_diag += I.T @ negm where negm is
     -240 on the strict lower triangle; exp(x/8) then underflows masked
     entries to f16 zero.  ONE exp over all 768 cols on ACT (scale=0.125;
     no max subtraction: scores ~ N(0,1), softmax is shift-invariant).
  7. out_unnorm[q,:] = p_block.T @ [v|1] accumulated over causal k blocks;
     col 64 = softmax denominator.  One PSUM tile [128,4,65] per group.
  8. One reciprocal + one broadcast multiply per group normalizes all four
     q-blocks, writing f16; one DMA stores the pair.  Host casts f32.

The AV/normalize/store tail of group g is EMITTED after the head of group
g+1: the Tile scheduler orders each engine's in-order queue by emission
priority, and the tail ops (which wait on exp) must not block the next
group's early DVE/PE work (head-of-line).  The first x-load is issued
before everything else; the weights load raw f32 over HWDGE (sync) and
are cast on DVE so the GpSimd descriptor queue stays clear for x loads.
"""

import numpy as np

import concourse.bass as bass
import concourse.mybir as mybir
import concourse.tile as tile
from concourse import bacc
from concourse.bass import ds, ts
from concourse.bass_utils import run_bass_kernel_spmd
from concourse.masks import make_identity, make_upper_triangular

EMBED = 384
HEAD_DIM = 64
SEQ = 256
BATCH = 512
NCORES = 8
NB = BATCH // NCORES  # batches per core

F32 = mybir.dt.float32
F16 = mybir.dt.float16
F8 = mybir.dt.float8e4
DR = mybir.MatmulPerfMode.DoubleRow

EC = EMBED // 128  # 3 embed chunks
ST = SEQ // 128    # 2 seq blocks


def _build(nb=NB):
    """Build the per-core Bass program for nb batches (processed in pairs)."""
    assert nb % 2 == 0
    GB = 2               # batches per group
    GS = GB * SEQ        # 512: grouped seq columns
    ng = nb // GB

    nc = bacc.Bacc("TRN2", target_bir_lowering=False)
    x = nc.dram_tensor("x", [nb, SEQ, EMBED], F32, kind="ExternalInput")
    wq = nc.dram_tensor("wq", [EMBED, HEAD_DIM], F32, kind="ExternalInput")
    wk = nc.dram_tensor("wk", [EMBED, HEAD_DIM], F32, kind="ExternalInput")
    wv = nc.dram_tensor("wv", [EMBED, HEAD_DIM], F32, kind="ExternalInput")
    out = nc.dram_tensor("out", [nb, SEQ, HEAD_DIM], F16, kind="ExternalOutput")

    xv = x[:, :, :].rearrange("(g b) (t p) e -> g p b t e", b=GB, p=128)
    ov = out[:, :, :].rearrange("(g b) (t p) d -> g p b t d", b=GB, p=128)

    AW = HEAD_DIM + 1   # 65: v columns + ones column

    with tile.TileContext(nc) as tc:
        with (
            tc.tile_pool(name="const", bufs=1) as cpool,
            tc.tile_pool(name="sb_x", bufs=6) as sb_x,
            tc.tile_pool(name="sb_xt", bufs=4) as sb_xt,
            tc.tile_pool(name="sb_qk", bufs=4) as sb_qk,
            tc.tile_pool(name="sb_v", bufs=4) as sb_v,
            tc.tile_pool(name="sb_p", bufs=4) as sb_p,
            tc.tile_pool(name="sb_o", bufs=4) as sb_o,
            tc.tile_pool(name="ps_xt", bufs=1, space="PSUM") as ps_xt,
            tc.tile_pool(name="ps_kvq", bufs=2, space="PSUM") as ps_kvq,
            tc.tile_pool(name="ps_st", bufs=1, space="PSUM") as ps_st,
            tc.tile_pool(name="ps_misc", bufs=1, space="PSUM") as ps_misc,
        ):
            # First x-load ahead of everything: the transposes need it
            # before any weights are needed.
            xs0 = sb_x.tile([128, GB, ST, EMBED], F16, tag="xs")
            nc.gpsimd.dma_start(out=xs0[:, :, :, :], in_=xv[0])

            ident = cpool.tile([128, 128], F16)
            make_identity(nc, ident)
            # tri[k, q] = 1.0 where k <= q else 0.0 (multiplied onto the
            # diagonal blocks of exp(scores) on GpSimd)
            tri = cpool.tile([128, 128], F16)
            make_upper_triangular(nc, tri, val=1.0, diag=True)
            tri_b2 = bass.AP(
                tensor=tri.tensor,
                offset=tri.offset,
                ap=[tri.ap[0], [0, 2], [1, 128]],
            )

            # Weights: raw f32 over HWDGE (keeps GpSimd free for x loads),
            # then DVE-cast to fp8.  [Wk|Wv] packed: kT lands at partitions
            # 0:64 (base 0, as the scores matmul needs), vT at 64:128 (only
            # feeds the PE transpose at base 64).  Wq zero-padded to 128
            # columns (its junk output rows 64:128 are never read).
            w32 = cpool.tile([128, EC, 3, HEAD_DIM], F32)
            nc.sync.dma_start(
                out=w32[:, :, 0, :], in_=wk[:, :].rearrange("(c p) d -> p c d", p=128)
            )
            nc.sync.dma_start(
                out=w32[:, :, 1, :], in_=wv[:, :].rearrange("(c p) d -> p c d", p=128)
            )
            nc.sync.dma_start(
                out=w32[:, :, 2, :], in_=wq[:, :].rearrange("(c p) d -> p c d", p=128)
            )
            wkv_sb = cpool.tile([128, EC, 128], F16)
            wq_sb = cpool.tile([128, EC, 128], F16)
            nc.vector.tensor_copy(
                wkv_sb[:, :, :].rearrange("p c (i d) -> p c i d", i=2),
                w32[:, :, 0:2, :],
            )
            nc.vector.memset(wq_sb[:, :, HEAD_DIM:128], 0.0)
            nc.vector.tensor_copy(wq_sb[:, :, 0:HEAD_DIM], w32[:, :, 2, :])

            def head_a(g):
                """Load, transpose, project, cast for group g."""
                if g == 0:
                    xs = xs0
                else:
                    xs = sb_x.tile([128, GB, ST, EMBED], F16, tag="xs")
                    nc.gpsimd.dma_start(out=xs[:, :, :, :], in_=xv[g])

                # transpose x -> xT; block (b,t,c) at col c*512+b*256+t*128.
                # Two rounds of 6 blocks through a 1-bank staging tile (the
                # freed PSUM bank pays for double-buffering the projections).
                xts = sb_xt.tile([128, EC, GS], F16, tag="xts")
                xts_flat = xts[:, :, :].rearrange("p c s -> p (c s)")
                blocks = [
                    (c, b, t)
                    for c in range(EC)
                    for b in range(GB)
                    for t in range(ST)
                ]
                for r in range(2):
                    xt_ps = ps_xt.tile([128, 768], F16, tag="xt")
                    for j, (c, b, t) in enumerate(blocks[r * 6 : r * 6 + 6]):
                        nc.tensor.transpose(
                            xt_ps[:, ds(j * 128, 128)],
                            xs[:, b, t, ts(c, 128)],
                            ident[:, :],
                        )
                    nc.vector.tensor_copy(
                        xts_flat[:, ds(r * 768, 768)], xt_ps[:, :]
                    )

                # [kT; vT] (cols 0:512) and qT (cols 512:1024) projections
                kvq_ps = ps_kvq.tile([128, 2 * GS], F32, tag="kvq")
                for c in range(EC):
                    nc.tensor.matmul(
                        kvq_ps[:, 0:GS], wkv_sb[:, c, :], xts[:, c, :],
                        start=(c == 0), stop=(c == EC - 1),
                    )
                for c in range(EC):
                    nc.tensor.matmul(
                        kvq_ps[:, GS : 2 * GS], wq_sb[:, c, :], xts[:, c, :],
                        start=(c == 0), stop=(c == EC - 1),
                    )
                # ONE f32->f16 cast for k, v and q together
                kvq_sb = sb_qk.tile([128, 2 * GS], F16, tag="kvq_sb")
                nc.vector.tensor_copy(kvq_sb[:, :], kvq_ps[:, :])
                return xs, kvq_sb

            def head_b(g, kvq_sb):
                """v transpose, scores, exp for group g."""

                def kT(b, tb):  # [64, 128] lhsT slice for k block tb
                    return kvq_sb[0:HEAD_DIM, ds(b * SEQ + tb * 128, 128)]

                def qT(b, lo, n):  # [64, n] rhs slice of q cols lo:lo+n
                    return kvq_sb[0:HEAD_DIM, ds(GS + b * SEQ + lo, n)]

                # transpose vT back to natural v; ACT copies beside the
                # ones column (col 64 -> softmax denominator)
                vn_ps = ps_misc.tile([128, GB * ST * HEAD_DIM], F16, tag="vn")
                for b in range(GB):
                    for t in range(ST):
                        nc.tensor.transpose(
                            vn_ps[:, ds((b * ST + t) * HEAD_DIM, HEAD_DIM)],
                            kvq_sb[HEAD_DIM:128, ds(b * SEQ + t * 128, 128)],
                            ident[HEAD_DIM:128, HEAD_DIM:128],
                        )
                v_sb = sb_v.tile([128, GB, ST, AW], F16, tag="v_sb")
                nc.scalar.copy(
                    v_sb[:, :, :, 0:HEAD_DIM],
                    vn_ps[:, :].rearrange("p (b t d) -> p b t d", b=GB, t=ST),
                )
                nc.vector.memset(v_sb[:, :, :, HEAD_DIM:AW], 1.0)

                # scores per batch through a 1-bank PSUM tile; pt half b at
                # cols b*384: [k0-span q0|q1 (256) | k1q1 (128)].
                # start=True clears has_written for the whole bank, so only
                # the first matmul per tile opens.
                pt_sb = sb_p.tile([128, GB, 384], F16, tag="pt")
                for b in range(GB):
                    st_ps = ps_st.tile([128, 384], F32, tag="st")
                    nc.tensor.matmul(
                        st_ps[:, 0:256], kT(b, 0), qT(b, 0, 256),
                        start=True, stop=False,
                    )
                    nc.tensor.matmul(
                        st_ps[:, 256:384], kT(b, 1), qT(b, 128, 128),
                        start=False, stop=True,
                    )
                    nc.scalar.activation(
                        pt_sb[:, b, :],
                        st_ps[:, :],
                        mybir.ActivationFunctionType.Exp,
                        scale=0.125,
                    )
                # GpSimd masks the four causal-diagonal blocks (cols 0:128,
                # the contiguous pair 256:512, and 640:768) — AV runs two
                # pipeline stages later, so this is slack work.
                ptf = pt_sb[:, :, :].rearrange("p b s -> p (b s)")
                nc.gpsimd.tensor_mul(ptf[:, 0:128], ptf[:, 0:128], tri[:, :])
                diag2 = ptf[:, 256:512].rearrange("p (i s) -> p i s", i=2)
                nc.gpsimd.tensor_mul(diag2, diag2, tri_b2)
                nc.gpsimd.tensor_mul(
                    ptf[:, 640:768], ptf[:, 640:768], tri[:, :]
                )
                return ptf, v_sb

            def tail(g, pt_sb, v_sb):
                """AV, normalize, store for group g."""
                av_ps = ps_misc.tile([128, 2 * GB, AW], F32, tag="av")
                nc.tensor.matmul(
                    av_ps[:, 0, :], pt_sb[:, 0:128], v_sb[:, 0, 0, :],
                    start=True, stop=True,
                )
                nc.tensor.matmul(
                    av_ps[:, 1, :], pt_sb[:, 128:256], v_sb[:, 0, 0, :],
                    start=True, stop=False,
                )
                nc.tensor.matmul(
                    av_ps[:, 1, :], pt_sb[:, 256:384], v_sb[:, 0, 1, :],
                    start=False, stop=True,
                )
                nc.tensor.matmul(
                    av_ps[:, 2, :], pt_sb[:, 384:512], v_sb[:, 1, 0, :],
                    start=True, stop=True,
                )
                nc.tensor.matmul(
                    av_ps[:, 3, :], pt_sb[:, 512:640], v_sb[:, 1, 0, :],
                    start=True, stop=False,
                )
                nc.tensor.matmul(
                    av_ps[:, 3, :], pt_sb[:, 640:768], v_sb[:, 1, 1, :],
                    start=False, stop=True,
                )

                out_sb = sb_o.tile([128, GB, ST, HEAD_DIM], F16, tag="out_sb")
                linv = sb_o.tile([128, 2 * GB], F32, tag="linv")
                nc.vector.reciprocal(
                    linv[:, :], av_ps[:, :, HEAD_DIM : HEAD_DIM + 1]
                )
                linv_b = bass.AP(
                    tensor=linv.tensor,
                    offset=linv.offset,
                    ap=[linv.ap[0], [1, 2 * GB], [0, HEAD_DIM]],
                )
                nc.vector.tensor_mul(
                    out_sb[:, :, :, :].rearrange("p b t d -> p (b t) d"),
                    av_ps[:, :, 0:HEAD_DIM],
                    linv_b,
                )
                nc.sync.dma_start(out=ov[g], in_=out_sb[:, :, :, :])

            # two-stage software pipeline: while group g's cast runs on DVE,
            # the PE works on group g+1's transposes/projections; the
            # exp->AV tail trails one further stage behind.
            a_pend = None   # (g, kvq_sb) awaiting head_b
            b_pend = None   # (g, pt_sb, v_sb) awaiting tail
            for g in range(ng):
                cur_a = (g, head_a(g)[1])
                if a_pend is not None:
                    ga, kvq_prev = a_pend
                    cur_b = (ga, *head_b(ga, kvq_prev))
                    if b_pend is not None:
                        tail(b_pend[0], b_pend[1], b_pend[2])
                    b_pend = cur_b
                a_pend = cur_a
            ga, kvq_prev = a_pend
            cur_b = (ga, *head_b(ga, kvq_prev))
            if b_pend is not None:
                tail(b_pend[0], b_pend[1], b_pend[2])
            tail(cur_b[0], cur_b[1], cur_b[2])

    nc.compile()
    return nc


_NC_CACHE = {}


def _get_nc(nb=NB):
    if nb not in _NC_CACHE:
        _NC_CACHE[nb] = _build(nb)
    return _NC_CACHE[nb]


def kernel(input_tensor, Wq, Wk, Wv, **run_kwargs):
    x = np.ascontiguousarray(np.asarray(input_tensor, dtype=np.float32))
    wq = np.ascontiguousarray(np.asarray(Wq, dtype=np.float32))
    wk = np.ascontiguousarray(np.asarray(Wk, dtype=np.float32))
    wv = np.ascontiguousarray(np.asarray(Wv, dtype=np.float32))

    nb = x.shape[0] // NCORES
    nc = _get_nc(nb=nb)
    in_maps = [
        {"x": x[i * nb : (i + 1) * nb], "wq": wq, "wk": wk, "wv": wv}
        for i in range(NCORES)
    ]
    res = run_bass_kernel_spmd(nc, in_maps, core_ids=list(range(NCORES)), **run_kwargs)
    outs = np.concatenate(
        [res.results[i]["out"] for i in range(NCORES)], axis=0
    ).astype(np.float32)
    if run_kwargs.get("trace"):
        kernel.last_results = res
    return outs


# revision 40
# speedup vs baseline: 1.2635x; 1.2635x over previous
"""Causal single-head attention on 8 Trainium2 NeuronCores.

Shapes (hardcoded per problem spec):
  input_tensor [512, 256, 384] f32, Wq/Wk/Wv [384, 64] f32 -> out [512, 256, 64] f32

Sharding: data-parallel on the batch dim, 64 batches per core, weights
replicated.

Per-batch-pair pipeline on each core (S=256 split into two 128-row blocks,
E=384 split into three 128-row chunks, GB=2 batches per group):
  1. DMA x pair [2,256,384] into SBUF with an f32->f16 cast (SWDGE).
  2. PE-transpose the twelve 128x128 blocks -> xT (f16 PSUM), DVE copies to
     SBUF in two pieces (chunks 0-1, then chunk 2) so the projections can
     start after the first piece.
  3. One PSUM tile [128,1024] holds both projections: [Wk|Wv] -> cols
     0:512 (kT at partitions 0:64, vT at 64:128), [Wq|0pad] -> cols
     512:1024 (the zero-pad makes the weights 128 wide so fast-weight-load
     engages).  DVE casts kv then q to f16 SBUF (split so the v-transposes
     unblock after the first).
  4. PE-transpose vT back to natural v [256,64]; ACT copies it beside a
     ones column (col 64 -> softmax denominator inside the AV matmul).
  5. Both batches' scores (f16 operands) in one PSUM tile [128,768], layout
     [b0k0(0:256) | b0k1(256:384) | b1k1(384:512) | b1k0(512:768)] so every
     matmul stays inside a 2KB PSUM bank.  start=True clears has_written
     for the WHOLE bank, so only the first matmul per bank opens; the rest
     accumulate/overwrite per element with start=False.
  6. Causal mask as PE accumulate: st_diag += I.T @ negm where negm is
     -240 on the strict lower triangle; exp(x/8) then underflows masked
     entries to f16 zero.  ONE exp over all 768 cols on ACT (scale=0.125;
     no max subtraction: scores ~ N(0,1), softmax is shift-invariant).
  7. out_unnorm[q,:] = p_block.T @ [v|1] accumulated over causal k blocks;
     col 64 = softmax denominator.  One PSUM tile [128,4,65] per group.
  8. One reciprocal + one broadcast multiply per group normalizes all four
     q-blocks, writing f16; one DMA stores the pair.  Host casts f32.

The AV/normalize/store tail of group g is EMITTED after the head of group
g+1: the Tile scheduler orders each engine's in-order queue by emission
priority, and the tail ops (which wait on exp) must not block the next
group's early DVE/PE work (head-of-line).  The first x-load is issued
before everything else; the weights load raw f32 over HWDGE (sync) and
are cast on DVE so the GpSimd descriptor queue stays clear for x loads.
"""

import numpy as np

import concourse.bass as bass
import concourse.mybir as mybir
import concourse.tile as tile
from concourse import bacc
from concourse.bass import ds, ts
from concourse.bass_utils import run_bass_kernel_spmd
from concourse.masks import make_identity, make_lower_triangular

EMBED = 384
HEAD_DIM = 64
SEQ = 256
BATCH = 512
NCORES = 8
NB = BATCH // NCORES  # batches per core

F32 = mybir.dt.float32
F16 = mybir.dt.float16
F8 = mybir.dt.float8e4
DR = mybir.MatmulPerfMode.DoubleRow

EC = EMBED // 128  # 3 embed chunks
ST = SEQ // 128    # 2 seq blocks


def _build(nb=NB):
    """Build the per-core Bass program for nb batches (processed in pairs)."""
    assert nb % 2 == 0
    GB = 2               # batches per group
    GS = GB * SEQ        # 512: grouped seq columns
    ng = nb // GB

    nc = bacc.Bacc("TRN2", target_bir_lowering=False)
    x = nc.dram_tensor("x", [nb, SEQ, EMBED], F32, kind="ExternalInput")
    wq = nc.dram_tensor("wq", [EMBED, HEAD_DIM], F32, kind="ExternalInput")
    wk = nc.dram_tensor("wk", [EMBED, HEAD_DIM], F32, kind="ExternalInput")
    wv = nc.dram_tensor("wv", [EMBED, HEAD_DIM], F32, kind="ExternalInput")
    out = nc.dram_tensor("out", [nb, SEQ, HEAD_DIM], F16, kind="ExternalOutput")

    xv = x[:, :, :].rearrange("(g b) (t p) e -> g p b t e", b=GB, p=128)
    ov = out[:, :, :].rearrange("(g b) (t p) d -> g p b t d", b=GB, p=128)

    AW = HEAD_DIM + 1   # 65: v columns + ones column

    with tile.TileContext(nc) as tc:
        with (
            tc.tile_pool(name="const", bufs=1) as cpool,
            tc.tile_pool(name="sb_x", bufs=6) as sb_x,
            tc.tile_pool(name="sb_xt", bufs=4) as sb_xt,
            tc.tile_pool(name="sb_qk", bufs=4) as sb_qk,
            tc.tile_pool(name="sb_v", bufs=4) as sb_v,
            tc.tile_pool(name="sb_p", bufs=4) as sb_p,
            tc.tile_pool(name="sb_o", bufs=4) as sb_o,
            tc.tile_pool(name="ps_xt", bufs=1, space="PSUM") as ps_xt,
            tc.tile_pool(name="ps_kvq", bufs=1, space="PSUM") as ps_kvq,
            tc.tile_pool(name="ps_st", bufs=1, space="PSUM") as ps_st,
            tc.tile_pool(name="ps_misc", bufs=1, space="PSUM") as ps_misc,
        ):
            # First x-load ahead of everything, split per batch so the
            # first transposes start as soon as batch 0 lands (ramp only
            # matters for group 0 — later loads are fully prefetched).
            xs0 = sb_x.tile([128, GB, ST, EMBED], F16, tag="xs")
            nc.gpsimd.dma_start(out=xs0[:, 0, :, :], in_=xv[0, :, 0])
            nc.gpsimd.dma_start(out=xs0[:, 1, :, :], in_=xv[0, :, 1])

            ident = cpool.tile([128, 128], F16)
            make_identity(nc, ident)
            # negm[k, q] = -240 where k > q else 0: accumulated onto the
            # diagonal score blocks on the PE, the exp with scale 1/8 turns
            # masked entries into e^-30·p -> f16 zero.
            negm = cpool.tile([128, 128], F16)
            make_lower_triangular(nc, negm, val=-240.0, diag=False)

            # Weights: raw f32 over HWDGE (keeps GpSimd free for x loads),
            # then DVE-cast to fp8.  [Wk|Wv] packed: kT lands at partitions
            # 0:64 (base 0, as the scores matmul needs), vT at 64:128 (only
            # feeds the PE transpose at base 64).  Wq zero-padded to 128
            # columns (its junk output rows 64:128 are never read).
            w32 = cpool.tile([128, EC, 3, HEAD_DIM], F32)
            nc.sync.dma_start(
                out=w32[:, :, 0, :], in_=wk[:, :].rearrange("(c p) d -> p c d", p=128)
            )
            nc.sync.dma_start(
                out=w32[:, :, 1, :], in_=wv[:, :].rearrange("(c p) d -> p c d", p=128)
            )
            nc.sync.dma_start(
                out=w32[:, :, 2, :], in_=wq[:, :].rearrange("(c p) d -> p c d", p=128)
            )
            wkv_sb = cpool.tile([128, EC, 128], F16)
            wq_sb = cpool.tile([128, EC, 128], F16)
            nc.vector.tensor_copy(
                wkv_sb[:, :, :].rearrange("p c (i d) -> p c i d", i=2),
                w32[:, :, 0:2, :],
            )
            nc.vector.memset(wq_sb[:, :, HEAD_DIM:128], 0.0)
            nc.vector.tensor_copy(wq_sb[:, :, 0:HEAD_DIM], w32[:, :, 2, :])

            def head_a(g):
                """Load, transpose, project, cast for group g."""
                if g == 0:
                    xs = xs0
                else:
                    xs = sb_x.tile([128, GB, ST, EMBED], F16, tag="xs")
                    nc.gpsimd.dma_start(out=xs[:, :, :, :], in_=xv[g])

                # transpose x -> xT; block (b,t,c) at col c*512+b*256+t*128
                xt_ps = ps_xt.tile([128, EC * GS], F16, tag="xt")
                xts = sb_xt.tile([128, EC, GS], F16, tag="xts")
                if g == 0:
                    # batch-major with per-batch copies: batch 0's pipeline
                    # fills while batch 1's DMA is still landing
                    xtv = xts[:, :, :].rearrange("p c (b s) -> p c b s", b=GB)
                    for b in range(GB):
                        for c in range(EC):
                            for t in range(ST):
                                nc.tensor.transpose(
                                    xt_ps[:, ds(b * 768 + c * SEQ + t * 128, 128)],
                                    xs[:, b, t, ts(c, 128)],
                                    ident[:, :],
                                )
                        nc.vector.tensor_copy(
                            xtv[:, :, b, :],
                            xt_ps[:, ds(b * 768, 768)].rearrange(
                                "p (c s) -> p c s", c=EC
                            ),
                        )
                else:
                    for b in range(GB):
                        for t in range(ST):
                            for c in range(EC):
                                nc.tensor.transpose(
                                    xt_ps[:, ds(c * GS + b * SEQ + t * 128, 128)],
                                    xs[:, b, t, ts(c, 128)],
                                    ident[:, :],
                                )
                    nc.vector.tensor_copy(
                        xts[:, :, :],
                        xt_ps[:, :].rearrange("p (c s) -> p c s", c=EC),
                    )

                # [kT; vT] (cols 0:512) and qT (cols 512:1024) projections
                kvq_ps = ps_kvq.tile([128, 2 * GS], F32, tag="kvq")
                for c in range(EC):
                    nc.tensor.matmul(
                        kvq_ps[:, 0:GS], wkv_sb[:, c, :], xts[:, c, :],
                        start=(c == 0), stop=(c == EC - 1),
                    )
                for c in range(EC):
                    nc.tensor.matmul(
                        kvq_ps[:, GS : 2 * GS], wq_sb[:, c, :], xts[:, c, :],
                        start=(c == 0), stop=(c == EC - 1),
                    )
                # f32->f16 casts, kv piece first so the v-transposes (which
                # only need cols 0:512) unblock ~500ns earlier
                kvq_sb = sb_qk.tile([128, 2 * GS], F16, tag="kvq_sb")
                nc.vector.tensor_copy(kvq_sb[:, 0:GS], kvq_ps[:, 0:GS])
                nc.vector.tensor_copy(
                    kvq_sb[0:HEAD_DIM, GS : 2 * GS],
                    kvq_ps[0:HEAD_DIM, GS : 2 * GS],
                )
                return xs, kvq_sb

            def head_b(g, kvq_sb):
                """v transpose, scores, exp for group g."""

                def kT(b, tb):  # [64, 128] lhsT slice for k block tb
                    return kvq_sb[0:HEAD_DIM, ds(b * SEQ + tb * 128, 128)]

                def qT(b, lo, n):  # [64, n] rhs slice of q cols lo:lo+n
                    return kvq_sb[0:HEAD_DIM, ds(GS + b * SEQ + lo, n)]

                # transpose vT back to natural v; ACT copies beside the
                # ones column (col 64 -> softmax denominator)
                vn_ps = ps_misc.tile([128, GB * ST * HEAD_DIM], F16, tag="vn")
                for b in range(GB):
                    for t in range(ST):
                        nc.tensor.transpose(
                            vn_ps[:, ds((b * ST + t) * HEAD_DIM, HEAD_DIM)],
                            kvq_sb[HEAD_DIM:128, ds(b * SEQ + t * 128, 128)],
                            ident[HEAD_DIM:128, HEAD_DIM:128],
                        )
                v_sb = sb_v.tile([128, GB, ST, AW], F16, tag="v_sb")
                nc.scalar.copy(
                    v_sb[:, :, :, 0:HEAD_DIM],
                    vn_ps[:, :].rearrange("p (b t d) -> p b t d", b=GB, t=ST),
                )
                nc.vector.memset(v_sb[:, :, :, HEAD_DIM:AW], 1.0)

                # scores for BOTH batches in one PSUM tile:
                # [b0k0(0:256) | b0k1(256:384) | b1k1(384:512) | b1k0(512:768)]
                # one start=True opener per 2KB bank (clears has_written for
                # the whole bank), everything else start=False.
                st_ps = ps_st.tile([128, 768], F32, tag="st")
                nc.tensor.matmul(
                    st_ps[:, 0:256], kT(0, 0), qT(0, 0, 256),
                    start=True, stop=False,
                )
                nc.tensor.matmul(
                    st_ps[:, 256:384], kT(0, 1), qT(0, 128, 128),
                    start=False, stop=False,
                )
                nc.tensor.matmul(
                    st_ps[:, 384:512], kT(1, 1), qT(1, 128, 128),
                    start=False, stop=False,
                )
                nc.tensor.matmul(
                    st_ps[:, 512:768], kT(1, 0), qT(1, 0, 256),
                    start=True, stop=False,
                )
                # causal mask accumulate on the four diagonal blocks
                for off, stop in ((0, False), (256, False), (384, True), (512, True)):
                    nc.tensor.matmul(
                        st_ps[:, ds(off, 128)], ident[:, :], negm[:, :],
                        start=False, stop=stop,
                    )

                # ONE exp over both batches (masked entries underflow to 0)
                pt_sb = sb_p.tile([128, 768], F16, tag="pt")
                nc.scalar.activation(
                    pt_sb[:, :],
                    st_ps[:, :],
                    mybir.ActivationFunctionType.Exp,
                    scale=0.125,
                )
                return pt_sb, v_sb

            def tail(g, pt_sb, v_sb):
                """AV, normalize, store for group g."""
                av_ps = ps_misc.tile([128, 2 * GB, AW], F32, tag="av")
                nc.tensor.matmul(
                    av_ps[:, 0, :], pt_sb[:, 0:128], v_sb[:, 0, 0, :],
                    start=True, stop=True,
                )
                nc.tensor.matmul(
                    av_ps[:, 1, :], pt_sb[:, 128:256], v_sb[:, 0, 0, :],
                    start=True, stop=False,
                )
                nc.tensor.matmul(
                    av_ps[:, 1, :], pt_sb[:, 256:384], v_sb[:, 0, 1, :],
                    start=False, stop=True,
                )
                nc.tensor.matmul(
                    av_ps[:, 2, :], pt_sb[:, 512:640], v_sb[:, 1, 0, :],
                    start=True, stop=True,
                )
                nc.tensor.matmul(
                    av_ps[:, 3, :], pt_sb[:, 640:768], v_sb[:, 1, 0, :],
                    start=True, stop=False,
                )
                nc.tensor.matmul(
                    av_ps[:, 3, :], pt_sb[:, 384:512], v_sb[:, 1, 1, :],
                    start=False, stop=True,
                )

                out_sb = sb_o.tile([128, GB, ST, HEAD_DIM], F16, tag="out_sb")
                linv = sb_o.tile([128, 2 * GB], F32, tag="linv")
                nc.vector.reciprocal(
                    linv[:, :], av_ps[:, :, HEAD_DIM : HEAD_DIM + 1]
                )
                linv_b = bass.AP(
                    tensor=linv.tensor,
                    offset=linv.offset,
                    ap=[linv.ap[0], [1, 2 * GB], [0, HEAD_DIM]],
                )
                nc.vector.tensor_mul(
                    out_sb[:, :, :, :].rearrange("p b t d -> p (b t) d"),
                    av_ps[:, :, 0:HEAD_DIM],
                    linv_b,
                )
                nc.sync.dma_start(out=ov[g], in_=out_sb[:, :, :, :])

            # two-stage software pipeline: while group g's cast runs on DVE,
            # the PE works on group g+1's transposes/projections; the
            # exp->AV tail trails one further stage behind.
            a_pend = None   # (g, kvq_sb) awaiting head_b
            b_pend = None   # (g, pt_sb, v_sb) awaiting tail
            for g in range(ng):
                cur_a = (g, head_a(g)[1])
                if a_pend is not None:
                    ga, kvq_prev = a_pend
                    cur_b = (ga, *head_b(ga, kvq_prev))
                    if b_pend is not None:
                        tail(b_pend[0], b_pend[1], b_pend[2])
                    b_pend = cur_b
                a_pend = cur_a
            ga, kvq_prev = a_pend
            cur_b = (ga, *head_b(ga, kvq_prev))
            if b_pend is not None:
                tail(b_pend[0], b_pend[1], b_pend[2])
            tail(cur_b[0], cur_b[1], cur_b[2])

    nc.compile()
    return nc


_NC_CACHE = {}


def _get_nc(nb=NB):
    if nb not in _NC_CACHE:
        _NC_CACHE[nb] = _build(nb)
    return _NC_CACHE[nb]


def kernel(input_tensor, Wq, Wk, Wv, **run_kwargs):
    x = np.ascontiguousarray(np.asarray(input_tensor, dtype=np.float32))
    wq = np.ascontiguousarray(np.asarray(Wq, dtype=np.float32))
    wk = np.ascontiguousarray(np.asarray(Wk, dtype=np.float32))
    wv = np.ascontiguousarray(np.asarray(Wv, dtype=np.float32))

    nb = x.shape[0] // NCORES
    nc = _get_nc(nb=nb)
    in_maps = [
        {"x": x[i * nb : (i + 1) * nb], "wq": wq, "wk": wk, "wv": wv}
        for i in range(NCORES)
    ]
    res = run_bass_kernel_spmd(nc, in_maps, core_ids=list(range(NCORES)), **run_kwargs)
    outs = np.concatenate(
        [res.results[i]["out"] for i in range(NCORES)], axis=0
    ).astype(np.float32)
    if run_kwargs.get("trace"):
        kernel.last_results = res
    return outs
